# revision 9
# baseline (speedup 1.0000x reference)
"""3-layer GCN stack on 8 trn2 cores - v2 (pipelined piece AllGathers).

Changes vs v1:
- Table split into NP=4 "piece" tensors (one AllGather each, <=32768 rows so
  a gather call needs no block offset).  Piece AGs are emitted as soon as
  their windows flush, so the exchange streams during the layer instead of
  serializing at the layer boundary, and next-layer gathers start piece by
  piece.
- Unpadded cell stream: per-(window,piece) cells are padded only to the
  max-over-cores edge count (not to 128); matmul tiles may straddle cell /
  window boundaries and are consumed once per overlapping window.  The cs
  sentinel column is the cell's per-core real count (the natural cumsum
  tail), which makes ghost-gathered values cancel - no guaranteed-zero rows
  needed.  ~17% less gather DMA.
- Index stream and starts table are loaded to SBUF once (persist across
  layers) instead of re-DMAed per layer.
- Engine rebalance: suf PSUM->SBUF copy and the transform dis-scale run on
  ACT (activation Copy w/ per-partition scale) instead of DVE.
"""
import sys

if "/opt/trn_rl_repo" not in sys.path:
    sys.path.insert(0, "/opt/trn_rl_repo")

import numpy as np

import concourse.bacc as bacc
import concourse.bass as bass
import concourse.mybir as mybir
import concourse.tile as tile
from concourse.bass_utils import run_bass_kernel_spmd
from concourse.library_config import mlp as _mlp_lib

NCORES = 8
WIN = 127
NP = 4               # table pieces
CHUNK = 6144         # gather call granularity (indices)
G_SLOTS = 16
GMAX = 8             # max windows per group
SROWS = 32           # starts SBUF partition rows
F16 = mybir.dt.float16
F32 = mybir.dt.float32
I16 = mybir.dt.int16


def _ceil(a, b):
    return -(-a // b)


# --------------------------------------------------------------------------
# host-side preprocessing
# --------------------------------------------------------------------------

def _build_host(x, edge_index, batch):
    N, H = x.shape
    G = int(batch.max()) + 1 if batch.size else 1
    src = np.asarray(edge_index[0], dtype=np.int64)
    dst = np.asarray(edge_index[1], dtype=np.int64)
    batch = np.asarray(batch, dtype=np.int64)

    deg = np.bincount(dst, minlength=N).astype(np.float64) + 1.0
    dis = (1.0 / np.sqrt(deg)).astype(np.float32)

    # --- partition graphs -> cores (contiguous node ranges) ---
    gsizes = np.bincount(batch, minlength=G)
    gends = np.cumsum(gsizes)
    cuts = [0]
    for c in range(1, NCORES):
        target = round(N * c / NCORES)
        gi = min(int(np.searchsorted(gends, target)), G - 1)
        lo = int(gends[gi - 1]) if gi > 0 else 0
        hi = int(gends[gi])
        cut = lo if abs(lo - target) <= abs(hi - target) else hi
        cuts.append(max(cut, cuts[-1]))
    cuts.append(N)
    n0 = np.array(cuts[:-1], dtype=np.int64)
    n1 = np.array(cuts[1:], dtype=np.int64)
    counts = n1 - n0
    n_win = int(np.ceil(counts.max() / WIN))
    S_pad = n_win * WIN

    # --- pieces: contiguous window ranges, each <= 32768/ (8*WIN) windows ---
    maxw = 32768 // (NCORES * WIN)
    assert n_win <= NP * maxw, (n_win, NP, maxw)
    base, rem = divmod(n_win, NP)
    nw = np.array([base + (q < rem) for q in range(NP)], dtype=np.int64)
    w0 = np.concatenate([[0], np.cumsum(nw)])          # len NP+1
    piece_of_w = np.repeat(np.arange(NP), nw)
    rows_q = (NCORES * nw * WIN).astype(np.int64)      # piece table rows

    core_of = np.searchsorted(n1 - 1, np.arange(N), side="left")
    loc = np.arange(N) - n0[core_of]
    wv = loc // WIN
    qv = piece_of_w[wv]
    row_rel = core_of * nw[qv] * WIN + (loc - w0[qv] * WIN)
    assert row_rel.max() < 32768

    # --- per-core edge streams (edges + self loops, owned by dst core) ---
    all_src = np.concatenate([src, np.arange(N, dtype=np.int64)])
    all_dst = np.concatenate([dst, np.arange(N, dtype=np.int64)])
    e_core = core_of[all_dst]
    e_dloc = all_dst - n0[e_core]
    e_win = e_dloc // WIN
    e_q = qv[all_src]
    e_row = row_rel[all_src]

    # cell = (dst window, src piece); counts per core
    cnt = np.bincount(
        (e_core * n_win + e_win) * NP + e_q,
        minlength=NCORES * n_win * NP).reshape(NCORES, n_win, NP)
    cellcap = cnt.max(axis=0)                          # [n_win, NP]

    # template: piece-major regions; cells in window order, unpadded
    cell_off = np.zeros((n_win, NP), dtype=np.int64)
    piece_lo = np.zeros(NP, dtype=np.int64)
    piece_hi = np.zeros(NP, dtype=np.int64)
    off = 0
    for q in range(NP):
        piece_lo[q] = off
        for w in range(n_win):
            cell_off[w, q] = off
            off += int(cellcap[w, q])
        off = _ceil(off, 128) * 128
        piece_hi[q] = off
    TOT_IDX = int(off)

    # scatter edges into the template
    order = np.lexsort((e_dloc, e_win, e_q, e_core))
    s_core = e_core[order]
    s_q = e_q[order]
    s_win = e_win[order]
    s_dloc = e_dloc[order]
    s_row = e_row[order]
    s_seg = (s_core * NP + s_q) * n_win + s_win
    seg_first = np.concatenate([[True], s_seg[1:] != s_seg[:-1]])
    first_pos = np.flatnonzero(seg_first)
    run_id = np.cumsum(seg_first) - 1
    rank = np.arange(s_seg.size) - first_pos[run_id]
    pos = cell_off[s_win, s_q] + rank
    idx_stream = np.zeros((NCORES, TOT_IDX), dtype=np.int16)
    idx_stream[s_core, pos] = s_row.astype(np.int16)

    # per-dst-slot counts -> starts columns (cumsum; tail = cell count)
    cnt_dst = np.bincount(
        ((e_core * n_win + e_win) * NP + e_q) * WIN + (e_dloc % WIN),
        minlength=NCORES * n_win * NP * WIN
    ).reshape(NCORES, n_win, NP, WIN)
    starts_col = np.concatenate(
        [np.zeros((NCORES, n_win, NP, 1), np.int64),
         np.cumsum(cnt_dst, axis=3)], axis=3)          # [..., WIN+1]

    # groups: per piece, windows in chunks (balanced, <= GMAX)
    wgroups = []
    grp_piece = []
    for q in range(NP):
        nq = int(nw[q])
        ng = _ceil(nq, GMAX)
        sizes = [nq // ng + (i < nq % ng) for i in range(ng)]
        s = int(w0[q])
        for gi, sz in enumerate(sizes):
            wgroups.append(list(range(s, s + sz)))
            grp_piece.append(q if gi == ng - 1 else -1)  # AG after last group
            s += sz

    # per-window (piece, tile) pair list, in stream order
    wt_pairs = []
    for w in range(n_win):
        pl = []
        for q in range(NP):
            cap = int(cellcap[w, q])
            if cap == 0:
                continue
            o = int(cell_off[w, q])
            for t in range(o // 128, (o + cap - 1) // 128 + 1):
                pl.append((q, t))
        wt_pairs.append(pl)

    # starts values per (w, q, t) pair, in group-major consumption order
    pair_list = []                     # flat (w, q, t)
    pair_of = {}
    for grp in wgroups:
        for w in grp:
            for (q, t) in wt_pairs[w]:
                pair_of[(w, q, t)] = len(pair_list)
                pair_list.append((w, q, t))
    TOT_PAIRS = len(pair_list)
    per_row = _ceil(TOT_PAIRS, SROWS)
    SROW_COLS = per_row * 128

    starts = np.zeros((NCORES, SROWS, SROW_COLS), dtype=np.float16)
    for j, (w, q, t) in enumerate(pair_list):
        beta = t * 128 - int(cell_off[w, q])
        v = starts_col[:, w, q, :] - beta              # [NCORES, WIN+1]
        v = np.clip(v, -2, 130).astype(np.float16)
        r, cc = divmod(j, per_row)
        starts[:, r, cc * 128:(cc + 1) * 128] = v

    # cs-build chunks per group: runs of <=4 pairs within one starts row
    cs_chunks = []                     # per group: list of (row, col, js)
    jpos = 0
    for grp in wgroups:
        npairs = sum(len(wt_pairs[w]) for w in grp)
        chunks = []
        left = npairs
        while left:
            r, cc = divmod(jpos, per_row)
            k = min(4, left, per_row - cc)
            chunks.append((r, cc, list(range(jpos, jpos + k))))
            jpos += k
            left -= k
        cs_chunks.append(chunks)

    # gather chunk calls: per piece, window-ordered
    calls = []                         # (q, lo, hi)
    for q in range(NP):
        p = int(piece_lo[q])
        while p < int(piece_hi[q]):
            hh = min(p + CHUNK, int(piece_hi[q]))
            calls.append((q, p, hh))
            p = hh

    # wrapped idx layout [128, TOT_IDX/16]
    idx_wrapped = idx_stream.reshape(NCORES, TOT_IDX // 16, 16).transpose(0, 2, 1)
    idx_wrapped = np.tile(idx_wrapped, (1, 8, 1))

    # dis arrays
    dis_pad = np.zeros((NCORES, n_win, 128), dtype=np.float32)
    for c in range(NCORES):
        dv = dis[n0[c]:n1[c]]
        lv = np.arange(counts[c])
        dis_pad[c, lv // WIN, lv % WIN] = dv

    # pooling CS + recip
    first_graph = batch[np.minimum(n0, N - 1)]
    cs_pool = np.zeros((NCORES, n_win * 128, G_SLOTS), dtype=np.float16)
    recip = np.zeros((NCORES, G_SLOTS, 1), dtype=np.float32)
    gcount = np.bincount(batch, minlength=G).astype(np.float64)
    g_of_core = [[] for _ in range(NCORES)]
    for c in range(NCORES):
        gl = batch[n0[c]:n1[c]]
        if gl.size == 0:
            continue
        slots = gl - first_graph[c]
        assert slots.max() < G_SLOTS, "too many graphs on one core"
        lv = np.arange(counts[c])
        cs_pool[c, (lv // WIN) * 128 + (lv % WIN), slots] = 1.0
        for g in range(int(gl.min()), int(gl.max()) + 1):
            g_of_core[c].append(g)
            recip[c, g - first_graph[c], 0] = 1.0 / max(gcount[g], 1.0)

    # x fp16 padded [S_pad + 1, H] per core
    x_pad = np.zeros((NCORES, S_pad + 1, H), dtype=np.float16)
    for c in range(NCORES):
        x_pad[c, :counts[c]] = x[n0[c]:n1[c]].astype(np.float16)

    host = dict(
        H=H, G=G, n_win=n_win, S_pad=S_pad, TOT_IDX=TOT_IDX,
        TOT_PAIRS=TOT_PAIRS, per_row=per_row, SROW_COLS=SROW_COLS,
        nw=nw, w0=w0, rows_q=rows_q, piece_lo=piece_lo, piece_hi=piece_hi,
        wgroups=wgroups, grp_piece=grp_piece, wt_pairs=wt_pairs,
        pair_of=pair_of, cs_chunks=cs_chunks, calls=calls,
        n0=n0, n1=n1, counts=counts, first_graph=first_graph,
        g_of_core=g_of_core,
    )
    cs_poolT = cs_pool.reshape(NCORES, n_win, 128, G_SLOTS) \
        .transpose(0, 2, 1, 3).reshape(NCORES, 128, n_win * G_SLOTS)
    per_core = [
        dict(
            xin=np.ascontiguousarray(x_pad[c]),
            idxs=np.ascontiguousarray(idx_wrapped[c]),
            starts=np.ascontiguousarray(starts[c]),
            dis_row=np.ascontiguousarray(
                dis_pad[c].astype(np.float16).reshape(1, -1)),
            dis_colt=np.ascontiguousarray(dis_pad[c].T),
            cs_pool=np.ascontiguousarray(cs_poolT[c]),
            recip=np.ascontiguousarray(recip[c]),
        )
        for c in range(NCORES)
    ]
    return host, per_core


# --------------------------------------------------------------------------
# device program
# --------------------------------------------------------------------------

def _build_program(hp, L, single_core=False, nq=2):
    H = hp["H"]
    n_win, S_pad = hp["n_win"], hp["S_pad"]
    TOT_IDX = hp["TOT_IDX"]
    per_row, SROW_COLS = hp["per_row"], hp["SROW_COLS"]
    nw, w0, rows_q = hp["nw"], hp["w0"], hp["rows_q"]
    piece_lo, piece_hi = hp["piece_lo"], hp["piece_hi"]
    wgroups, grp_piece = hp["wgroups"], hp["grp_piece"]
    wt_pairs, pair_of = hp["wt_pairs"], hp["pair_of"]
    cs_chunks, calls = hp["cs_chunks"], hp["calls"]

    nc = bacc.Bacc("TRN2", target_bir_lowering=False, debug=False,
                   num_devices=1 if single_core else NCORES,
                   num_swdge_queues=nq)

    xin_d = nc.dram_tensor("xin", [S_pad + 1, H], F16, kind="ExternalInput")
    idx_d = nc.dram_tensor("idxs", [128, TOT_IDX // 16], I16, kind="ExternalInput")
    starts_d = nc.dram_tensor("starts", [SROWS, SROW_COLS], F16, kind="ExternalInput")
    disrow_d = nc.dram_tensor("dis_row", [1, n_win * 128], F16, kind="ExternalInput")
    discolt_d = nc.dram_tensor("dis_colt", [128, n_win], F32, kind="ExternalInput")
    cspool_d = nc.dram_tensor("cs_pool", [128, n_win * G_SLOTS], F16, kind="ExternalInput")
    recip_d = nc.dram_tensor("recip", [G_SLOTS, 1], F32, kind="ExternalInput")
    iota_d = nc.dram_tensor("iota", [128, 1], F16, kind="ExternalInput")
    iotasig_d = nc.dram_tensor("iotasig", [128, 1], F32, kind="ExternalInput")
    onehot_d = nc.dram_tensor("onehot", [SROWS, SROWS * 128], F16,
                              kind="ExternalInput")
    ident_d = nc.dram_tensor("ident", [H, H], F16, kind="ExternalInput")
    w_d = nc.dram_tensor("w", [L * H, H], F16, kind="ExternalInput")
    bias_d = nc.dram_tensor("bias", [L * H, 1], F32, kind="ExternalInput")
    out_d = nc.dram_tensor("out", [G_SLOTS, H], F32, kind="ExternalOutput")

    with tile.TileContext(nc) as tc:
        with tc.tile_pool(name="const", bufs=1) as cp, \
             tc.tile_pool(name="dram", bufs=1, space="DRAM") as dp, \
             tc.tile_pool(name="msg", bufs=2) as mp, \
             tc.tile_pool(name="sb", bufs=3) as sp, \
             tc.tile_pool(name="ps", bufs=2, space="PSUM") as pp, \
             tc.tile_pool(name="pool_ps", bufs=1, space="PSUM") as ppool:

            nc.gpsimd.load_library(_mlp_lib)

            iota_t = cp.tile([128, 1], F16)
            nc.sync.dma_start(iota_t[:], iota_d[:])
            iotasig_t = cp.tile([128, 1], F32)
            nc.sync.dma_start(iotasig_t[:], iotasig_d[:])
            ident_t = cp.tile([H, H], F16)
            nc.sync.dma_start(ident_t[:], ident_d[:])
            ones16 = cp.tile([1, 128], F16)
            nc.vector.memset(ones16[:], 1.0)
            onehot_t = cp.tile([SROWS, SROWS * 128], F16)
            nc.sync.dma_start(onehot_t[:], onehot_d[:])
            w_tiles, bias_tiles = [], []
            for l in range(L):
                wt = cp.tile([H, H], F16, tag=f"w{l}")
                nc.sync.dma_start(wt[:], w_d[l * H:(l + 1) * H, :])
                w_tiles.append(wt)
                bt = cp.tile([H, 1], F32, tag=f"b{l}")
                nc.sync.dma_start(bt[:], bias_d[l * H:(l + 1) * H, :])
                bias_tiles.append(bt)
            recip_t = cp.tile([G_SLOTS, 1], F32)
            nc.sync.dma_start(recip_t[:], recip_d[:])
            discolt_t = cp.tile([128, n_win], F32)
            nc.sync.dma_start(discolt_t[:], discolt_d[:])
            disrow_t = cp.tile([1, n_win * 128], F16)
            nc.sync.dma_start(disrow_t[:], disrow_d[:])
            cspool_t = cp.tile([128, n_win * G_SLOTS], F16)
            nc.sync.dma_start(cspool_t[:], cspool_d[:])
            # persistent: starts table + idx stream
            stg = cp.tile([SROWS, SROW_COLS], F16)
            nc.sync.dma_start(stg[:], starts_d[:])
            idxs_t = cp.tile([128, TOT_IDX // 16], I16)
            nc.sync.dma_start(idxs_t[:], idx_d[:])

            shard_a = dp.tile([S_pad, H], F16)
            shard_b = dp.tile([S_pad, H], F16)
            tspace = "Local" if single_core else "Shared"
            tbls = [[dp.tile([int(rows_q[q]), H], F16, addr_space=tspace,
                             name=f"tbl{l}_{q}") for q in range(NP)]
                    for l in range(L)]

            def transform(hT_f16, w_l, w, tq, qi):
                tps = pp.tile([WIN, H], F32, space="PSUM", tag="tps")
                nc.tensor.matmul(out=tps[:], lhsT=hT_f16[:, :WIN], rhs=w_l[:],
                                 start=True, stop=True)
                nc.scalar.activation(out=tq[:, qi, :], in_=tps[:],
                                     func=mybir.ActivationFunctionType.Copy,
                                     scale=discolt_t[0:WIN, w:w + 1])

            def flush_group(tq, grp, shard):
                gw0, glen = grp[0], len(grp)
                dst = shard[gw0 * WIN:(gw0 + glen) * WIN, :].rearrange(
                    "(q p) h -> p q h", p=WIN)
                nc.sync.dma_start(dst, tq[:, :glen, :])

            def emit_ag(q, shard, tbl_row):
                if single_core:
                    nc.sync.dma_start(
                        tbl_row[q][0:int(nw[q]) * WIN, :],
                        shard[int(w0[q]) * WIN:int(w0[q] + nw[q]) * WIN, :])
                else:
                    nc.gpsimd.collective_compute(
                        "AllGather", mybir.AluOpType.bypass,
                        replica_groups=[list(range(NCORES))],
                        ins=[shard[int(w0[q]) * WIN:int(w0[q] + nw[q]) * WIN, :]],
                        outs=[tbl_row[q][:]])

            # ---------------- layer 0 prologue: t0 = dis*(x@W0) --------------
            for gi, grp in enumerate(wgroups):
                tq = sp.tile([WIN, GMAX, H], F16, tag="t_sb")
                for qi, w in enumerate(grp):
                    xT = sp.tile([H, 128], F16, tag="xT")
                    nc.sync.dma_start(xT[:], xin_d[w * WIN:w * WIN + 128, :],
                                      transpose=True)
                    transform(xT, w_tiles[0], w, tq, qi)
                flush_group(tq, grp, shard_a)
                if grp_piece[gi] >= 0:
                    emit_ag(grp_piece[gi], shard_a, tbls[0])

            # ---------------- layers ----------------
            pool_ps = ppool.tile([G_SLOTS, H], F32, space="PSUM")
            n_pool_mm = 0

            for l in range(L):
                last = l == L - 1
                tbl_row = tbls[l]
                nshard = shard_b if l % 2 == 0 else shard_a
                ntbl = tbls[l + 1] if not last else None

                chunk_tiles = []
                for ci, (q, lo, hi) in enumerate(calls):
                    n = hi - lo
                    mt = mp.tile([128, n // 128, H], F16, tag=f"m{q}")
                    nc.gpsimd.dma_gather(
                        mt[:], tbl_row[q][:],
                        idxs_t[:, lo // 16:hi // 16], n, n, H,
                        single_packet=False, queue_num=ci % nq)
                    chunk_tiles.append((lo, hi, mt))

                def m_tile(p):
                    for lo, hi, mt in chunk_tiles:
                        if lo <= p < hi:
                            return mt[:, (p - lo) // 128, :]
                    raise AssertionError(p)

                csalt = 0
                for gi, grp in enumerate(wgroups):
                    cs_of = {}
                    for (r, cc, js) in cs_chunks[gi]:
                        k = len(js)
                        gn = k * 128
                        bps = pp.tile([128, 512], F32, space="PSUM", tag="bps")
                        nc.tensor.matmul(
                            out=bps[:, :gn],
                            lhsT=onehot_t[:, r * 128:(r + 1) * 128],
                            rhs=stg[0:SROWS, cc * 128:cc * 128 + gn],
                            start=True, stop=True)
                        cs = sp.tile([128, 512], F16, tag="cs", bufs=8)
                        if csalt % 2 == 0:
                            nc.vector.tensor_tensor(
                                out=cs[:, :gn],
                                in0=iota_t[:].to_broadcast([128, gn]),
                                in1=bps[:, :gn],
                                op=mybir.AluOpType.is_ge)
                        else:
                            nc.scalar.activation(
                                out=cs[:, :gn], in_=bps[:, :gn],
                                func=mybir.ActivationFunctionType.Sigmoid,
                                bias=iotasig_t[:], scale=-64.0)
                        csalt += 1
                        for ki, j in enumerate(js):
                            cs_of[j] = cs[:, ki * 128:(ki + 1) * 128]

                    tq = None
                    if not last:
                        tq = sp.tile([WIN, GMAX, H], F16, tag="t_sb")

                    for qi, w in enumerate(grp):
                        pairs = wt_pairs[w]
                        a_t = sp.tile([H, WIN], F32, tag="a", bufs=4)
                        if pairs:
                            suf = pp.tile([H, 128], F32, space="PSUM",
                                          tag="suf")
                            for k, (q, t) in enumerate(pairs):
                                nc.tensor.matmul(
                                    out=suf[:], lhsT=m_tile(t * 128),
                                    rhs=cs_of[pair_of[(w, q, t)]],
                                    start=(k == 0), stop=(k == len(pairs) - 1))
                            suf_sb = sp.tile([H, 128], F32, tag="suf_sb")
                            nc.scalar.activation(
                                out=suf_sb[:], in_=suf[:],
                                func=mybir.ActivationFunctionType.Copy)
                            nc.vector.tensor_tensor(out=a_t[:],
                                                    in0=suf_sb[:, :WIN],
                                                    in1=suf_sb[:, 1:WIN + 1],
                                                    op=mybir.AluOpType.subtract)
                        else:
                            nc.vector.memset(a_t[:], 0.0)

                        disb = pp.tile([128, WIN], F32, space="PSUM",
                                       tag="tps")
                        nc.tensor.matmul(out=disb[:], lhsT=ones16[:],
                                         rhs=disrow_t[:,
                                                      w * 128:w * 128 + WIN],
                                         start=True, stop=True)
                        u_t = sp.tile([H, WIN], F32, tag="u")
                        nc.vector.tensor_tensor(out=u_t[:], in0=a_t[:],
                                                in1=disb[:],
                                                op=mybir.AluOpType.mult)
                        hT = sp.tile([H, WIN], F16, tag="hT")
                        nc.scalar.activation(
                            out=hT[:], in_=u_t[:],
                            func=mybir.ActivationFunctionType.Relu,
                            bias=bias_tiles[l][:], scale=1.0)
                        if not last:
                            transform(hT, w_tiles[l + 1], w, tq, qi)
                        else:
                            hnm_ps = pp.tile([WIN, H], F32, space="PSUM",
                                             tag="tps")
                            nc.tensor.matmul(out=hnm_ps[:], lhsT=hT[:, :WIN],
                                             rhs=ident_t[:], start=True,
                                             stop=True)
                            hnm = sp.tile([WIN, H], F16, tag="hnm")
                            nc.vector.tensor_copy(hnm[:], hnm_ps[:])
                            nc.tensor.matmul(
                                out=pool_ps[:],
                                lhsT=cspool_t[0:WIN,
                                              w * G_SLOTS:(w + 1) * G_SLOTS],
                                rhs=hnm[:],
                                start=(n_pool_mm == 0),
                                stop=(n_pool_mm == n_win - 1))
                            n_pool_mm += 1
                    if not last:
                        flush_group(tq, grp, nshard)
                        if grp_piece[gi] >= 0:
                            emit_ag(grp_piece[gi], nshard, ntbl)

            pool_sb = sp.tile([G_SLOTS, H], F32, tag="pool_sb")
            nc.vector.tensor_scalar_mul(pool_sb[:], pool_ps[:], recip_t[:])
            nc.sync.dma_start(out_d[:], pool_sb[:])

    nc.compile()
    return nc


# --------------------------------------------------------------------------

def _full_in_maps(host, per_core, inputs):
    Ws = np.asarray(inputs["Ws"], dtype=np.float32)
    bs = np.asarray(inputs["bs"], dtype=np.float32)
    L, H = Ws.shape[0], Ws.shape[1]
    iota_col = np.arange(128, dtype=np.float16).reshape(128, 1)
    iotasig = (64.0 * np.arange(128) + 32.0).astype(np.float32).reshape(128, 1)
    ident = np.eye(H, dtype=np.float16)
    onehot = np.zeros((SROWS, SROWS * 128), dtype=np.float16)
    for r in range(SROWS):
        onehot[r, r * 128:(r + 1) * 128] = 1.0
    w_fp16 = np.ascontiguousarray(Ws.astype(np.float16).reshape(L * H, H))
    bias = np.ascontiguousarray(bs.astype(np.float32).reshape(L * H, 1))
    return [
        dict(pc, iota=iota_col, iotasig=iotasig, ident=ident, onehot=onehot,
             w=w_fp16, bias=bias)
        for pc in per_core
    ]


def kernel(x, edge_index, batch, Ws, bs):
    x = np.asarray(x)
    edge_index = np.asarray(edge_index)
    batch = np.asarray(batch)
    Ws = np.asarray(Ws, dtype=np.float32)
    bs = np.asarray(bs, dtype=np.float32)
    L, H = Ws.shape[0], Ws.shape[1]

    host, per_core = _build_host(x, edge_index, batch)
    nc = _build_program(host, L)
    in_maps = _full_in_maps(host, per_core, dict(Ws=Ws, bs=bs))
    res = run_bass_kernel_spmd(nc, in_maps, core_ids=list(range(NCORES)))

    G = host["G"]
    out = np.zeros((G, H), dtype=np.float32)
    for c in range(NCORES):
        fg = int(host["first_graph"][c])
        for g in host["g_of_core"][c]:
            out[g] = res.results[c]["out"][g - fg]
    return out


# revision 10
# speedup vs baseline: 37.3871x; 37.3871x over previous
"""3-layer GCN stack on 8 trn2 cores - v2 (pipelined piece AllGathers).

Changes vs v1:
- Table split into NP=4 "piece" tensors (one AllGather each, <=32768 rows so
  a gather call needs no block offset).  Piece AGs are emitted as soon as
  their windows flush, so the exchange streams during the layer instead of
  serializing at the layer boundary, and next-layer gathers start piece by
  piece.
- Unpadded cell stream: per-(window,piece) cells are padded only to the
  max-over-cores edge count (not to 128); matmul tiles may straddle cell /
  window boundaries and are consumed once per overlapping window.  The cs
  sentinel column is the cell's per-core real count (the natural cumsum
  tail), which makes ghost-gathered values cancel - no guaranteed-zero rows
  needed.  ~17% less gather DMA.
- Index stream and starts table are loaded to SBUF once (persist across
  layers) instead of re-DMAed per layer.
- Engine rebalance: suf PSUM->SBUF copy and the transform dis-scale run on
  ACT (activation Copy w/ per-partition scale) instead of DVE.
"""
import sys

if "/opt/trn_rl_repo" not in sys.path:
    sys.path.insert(0, "/opt/trn_rl_repo")

import numpy as np

import concourse.bacc as bacc
import concourse.bass as bass
import concourse.mybir as mybir
import concourse.tile as tile
from concourse.bass_utils import run_bass_kernel_spmd
from concourse.library_config import mlp as _mlp_lib

NCORES = 8
WIN = 127
NP = 4               # table pieces
CHUNK = 6144         # gather call granularity (indices)
G_SLOTS = 16
GMAX = 8             # max windows per group
SROWS = 32           # starts SBUF partition rows
F16 = mybir.dt.float16
F32 = mybir.dt.float32
I16 = mybir.dt.int16


def _ceil(a, b):
    return -(-a // b)


# --------------------------------------------------------------------------
# host-side preprocessing
# --------------------------------------------------------------------------

def _build_host(x, edge_index, batch):
    N, H = x.shape
    G = int(batch.max()) + 1 if batch.size else 1
    src = np.asarray(edge_index[0], dtype=np.int64)
    dst = np.asarray(edge_index[1], dtype=np.int64)
    batch = np.asarray(batch, dtype=np.int64)

    deg = np.bincount(dst, minlength=N).astype(np.float64) + 1.0
    dis = (1.0 / np.sqrt(deg)).astype(np.float32)

    # --- partition graphs -> cores (contiguous node ranges) ---
    gsizes = np.bincount(batch, minlength=G)
    gends = np.cumsum(gsizes)
    cuts = [0]
    for c in range(1, NCORES):
        target = round(N * c / NCORES)
        gi = min(int(np.searchsorted(gends, target)), G - 1)
        lo = int(gends[gi - 1]) if gi > 0 else 0
        hi = int(gends[gi])
        cut = lo if abs(lo - target) <= abs(hi - target) else hi
        cuts.append(max(cut, cuts[-1]))
    cuts.append(N)
    n0 = np.array(cuts[:-1], dtype=np.int64)
    n1 = np.array(cuts[1:], dtype=np.int64)
    counts = n1 - n0
    n_win = int(np.ceil(counts.max() / WIN))
    S_pad = n_win * WIN

    # --- pieces: contiguous window ranges, each <= 32768/ (8*WIN) windows ---
    maxw = 32768 // (NCORES * WIN)
    assert n_win <= NP * maxw, (n_win, NP, maxw)
    base, rem = divmod(n_win, NP)
    nw = np.array([base + (q < rem) for q in range(NP)], dtype=np.int64)
    w0 = np.concatenate([[0], np.cumsum(nw)])          # len NP+1
    piece_of_w = np.repeat(np.arange(NP), nw)
    rows_q = (NCORES * nw * WIN).astype(np.int64)      # piece table rows

    core_of = np.searchsorted(n1 - 1, np.arange(N), side="left")
    loc = np.arange(N) - n0[core_of]
    wv = loc // WIN
    qv = piece_of_w[wv]
    row_rel = core_of * nw[qv] * WIN + (loc - w0[qv] * WIN)
    assert row_rel.max() < 32768

    # --- per-core edge streams (edges + self loops, owned by dst core) ---
    all_src = np.concatenate([src, np.arange(N, dtype=np.int64)])
    all_dst = np.concatenate([dst, np.arange(N, dtype=np.int64)])
    e_core = core_of[all_dst]
    e_dloc = all_dst - n0[e_core]
    e_win = e_dloc // WIN
    e_q = qv[all_src]
    e_row = row_rel[all_src]

    # cell = (dst window, src piece); counts per core
    cnt = np.bincount(
        (e_core * n_win + e_win) * NP + e_q,
        minlength=NCORES * n_win * NP).reshape(NCORES, n_win, NP)
    cellcap = cnt.max(axis=0)                          # [n_win, NP]

    # template: piece-major regions; cells in window order, unpadded
    cell_off = np.zeros((n_win, NP), dtype=np.int64)
    piece_lo = np.zeros(NP, dtype=np.int64)
    piece_hi = np.zeros(NP, dtype=np.int64)
    off = 0
    for q in range(NP):
        piece_lo[q] = off
        for w in range(n_win):
            cell_off[w, q] = off
            off += int(cellcap[w, q])
        off = _ceil(off, 128) * 128
        piece_hi[q] = off
    TOT_IDX = int(off)

    # scatter edges into the template
    order = np.lexsort((e_dloc, e_win, e_q, e_core))
    s_core = e_core[order]
    s_q = e_q[order]
    s_win = e_win[order]
    s_dloc = e_dloc[order]
    s_row = e_row[order]
    s_seg = (s_core * NP + s_q) * n_win + s_win
    seg_first = np.concatenate([[True], s_seg[1:] != s_seg[:-1]])
    first_pos = np.flatnonzero(seg_first)
    run_id = np.cumsum(seg_first) - 1
    rank = np.arange(s_seg.size) - first_pos[run_id]
    pos = cell_off[s_win, s_q] + rank
    idx_stream = np.zeros((NCORES, TOT_IDX), dtype=np.int16)
    idx_stream[s_core, pos] = s_row.astype(np.int16)

    # per-dst-slot counts -> starts columns (cumsum; tail = cell count)
    cnt_dst = np.bincount(
        ((e_core * n_win + e_win) * NP + e_q) * WIN + (e_dloc % WIN),
        minlength=NCORES * n_win * NP * WIN
    ).reshape(NCORES, n_win, NP, WIN)
    starts_col = np.concatenate(
        [np.zeros((NCORES, n_win, NP, 1), np.int64),
         np.cumsum(cnt_dst, axis=3)], axis=3)          # [..., WIN+1]

    # groups: per piece, windows in chunks (balanced, <= GMAX)
    wgroups = []
    grp_piece = []
    for q in range(NP):
        nq = int(nw[q])
        ng = _ceil(nq, GMAX)
        sizes = [nq // ng + (i < nq % ng) for i in range(ng)]
        s = int(w0[q])
        for gi, sz in enumerate(sizes):
            wgroups.append(list(range(s, s + sz)))
            grp_piece.append(q if gi == ng - 1 else -1)  # AG after last group
            s += sz

    # per-window (piece, tile) pair list, in stream order
    wt_pairs = []
    for w in range(n_win):
        pl = []
        for q in range(NP):
            cap = int(cellcap[w, q])
            if cap == 0:
                continue
            o = int(cell_off[w, q])
            for t in range(o // 128, (o + cap - 1) // 128 + 1):
                pl.append((q, t))
        wt_pairs.append(pl)

    # starts values per (w, q, t) pair, in group-major consumption order
    pair_list = []                     # flat (w, q, t)
    pair_of = {}
    for grp in wgroups:
        for w in grp:
            for (q, t) in wt_pairs[w]:
                pair_of[(w, q, t)] = len(pair_list)
                pair_list.append((w, q, t))
    TOT_PAIRS = len(pair_list)
    per_row = _ceil(TOT_PAIRS, SROWS)
    SROW_COLS = per_row * 128

    starts = np.zeros((NCORES, SROWS, SROW_COLS), dtype=np.float16)
    for j, (w, q, t) in enumerate(pair_list):
        beta = t * 128 - int(cell_off[w, q])
        v = starts_col[:, w, q, :] - beta              # [NCORES, WIN+1]
        v = np.clip(v, -2, 130).astype(np.float16)
        r, cc = divmod(j, per_row)
        starts[:, r, cc * 128:(cc + 1) * 128] = v

    # cs-build chunks per group: runs of <=4 pairs within one starts row
    cs_chunks = []                     # per group: list of (row, col, js)
    jpos = 0
    for grp in wgroups:
        npairs = sum(len(wt_pairs[w]) for w in grp)
        chunks = []
        left = npairs
        while left:
            r, cc = divmod(jpos, per_row)
            k = min(4, left, per_row - cc)
            chunks.append((r, cc, list(range(jpos, jpos + k))))
            jpos += k
            left -= k
        cs_chunks.append(chunks)

    # gather chunk calls: per piece, window-ordered
    calls = []                         # (q, lo, hi)
    for q in range(NP):
        p = int(piece_lo[q])
        while p < int(piece_hi[q]):
            hh = min(p + CHUNK, int(piece_hi[q]))
            calls.append((q, p, hh))
            p = hh

    # wrapped idx layout [128, TOT_IDX/16]
    idx_wrapped = idx_stream.reshape(NCORES, TOT_IDX // 16, 16).transpose(0, 2, 1)
    idx_wrapped = np.tile(idx_wrapped, (1, 8, 1))

    # dis arrays
    dis_pad = np.zeros((NCORES, n_win, 128), dtype=np.float32)
    for c in range(NCORES):
        dv = dis[n0[c]:n1[c]]
        lv = np.arange(counts[c])
        dis_pad[c, lv // WIN, lv % WIN] = dv

    # pooling CS + recip
    first_graph = batch[np.minimum(n0, N - 1)]
    cs_pool = np.zeros((NCORES, n_win * 128, G_SLOTS), dtype=np.float16)
    recip = np.zeros((NCORES, G_SLOTS, 1), dtype=np.float32)
    gcount = np.bincount(batch, minlength=G).astype(np.float64)
    g_of_core = [[] for _ in range(NCORES)]
    for c in range(NCORES):
        gl = batch[n0[c]:n1[c]]
        if gl.size == 0:
            continue
        slots = gl - first_graph[c]
        assert slots.max() < G_SLOTS, "too many graphs on one core"
        lv = np.arange(counts[c])
        cs_pool[c, (lv // WIN) * 128 + (lv % WIN), slots] = 1.0
        for g in range(int(gl.min()), int(gl.max()) + 1):
            g_of_core[c].append(g)
            recip[c, g - first_graph[c], 0] = 1.0 / max(gcount[g], 1.0)

    # x fp16 padded [S_pad + 1, H] per core
    x_pad = np.zeros((NCORES, S_pad + 1, H), dtype=np.float16)
    for c in range(NCORES):
        x_pad[c, :counts[c]] = x[n0[c]:n1[c]].astype(np.float16)

    host = dict(
        H=H, G=G, n_win=n_win, S_pad=S_pad, TOT_IDX=TOT_IDX,
        TOT_PAIRS=TOT_PAIRS, per_row=per_row, SROW_COLS=SROW_COLS,
        nw=nw, w0=w0, rows_q=rows_q, piece_lo=piece_lo, piece_hi=piece_hi,
        wgroups=wgroups, grp_piece=grp_piece, wt_pairs=wt_pairs,
        pair_of=pair_of, cs_chunks=cs_chunks, calls=calls,
        n0=n0, n1=n1, counts=counts, first_graph=first_graph,
        g_of_core=g_of_core,
    )
    cs_poolT = cs_pool.reshape(NCORES, n_win, 128, G_SLOTS) \
        .transpose(0, 2, 1, 3).reshape(NCORES, 128, n_win * G_SLOTS)
    per_core = [
        dict(
            xin=np.ascontiguousarray(x_pad[c]),
            idxs=np.ascontiguousarray(idx_wrapped[c]),
            starts=np.ascontiguousarray(starts[c]),
            dis_row=np.ascontiguousarray(
                dis_pad[c].astype(np.float16).reshape(1, -1)),
            dis_colt=np.ascontiguousarray(dis_pad[c].T),
            cs_pool=np.ascontiguousarray(cs_poolT[c]),
            recip=np.ascontiguousarray(recip[c]),
        )
        for c in range(NCORES)
    ]
    return host, per_core


# --------------------------------------------------------------------------
# device program
# --------------------------------------------------------------------------

def _build_program(hp, L, single_core=False, nq=2):
    H = hp["H"]
    n_win, S_pad = hp["n_win"], hp["S_pad"]
    TOT_IDX = hp["TOT_IDX"]
    per_row, SROW_COLS = hp["per_row"], hp["SROW_COLS"]
    nw, w0, rows_q = hp["nw"], hp["w0"], hp["rows_q"]
    piece_lo, piece_hi = hp["piece_lo"], hp["piece_hi"]
    wgroups, grp_piece = hp["wgroups"], hp["grp_piece"]
    wt_pairs, pair_of = hp["wt_pairs"], hp["pair_of"]
    cs_chunks, calls = hp["cs_chunks"], hp["calls"]

    nc = bacc.Bacc("TRN2", target_bir_lowering=False, debug=False,
                   num_devices=1 if single_core else NCORES,
                   num_swdge_queues=nq)

    xin_d = nc.dram_tensor("xin", [S_pad + 1, H], F16, kind="ExternalInput")
    idx_d = nc.dram_tensor("idxs", [128, TOT_IDX // 16], I16, kind="ExternalInput")
    starts_d = nc.dram_tensor("starts", [SROWS, SROW_COLS], F16, kind="ExternalInput")
    disrow_d = nc.dram_tensor("dis_row", [1, n_win * 128], F16, kind="ExternalInput")
    discolt_d = nc.dram_tensor("dis_colt", [128, n_win], F32, kind="ExternalInput")
    cspool_d = nc.dram_tensor("cs_pool", [128, n_win * G_SLOTS], F16, kind="ExternalInput")
    recip_d = nc.dram_tensor("recip", [G_SLOTS, 1], F32, kind="ExternalInput")
    iota_d = nc.dram_tensor("iota", [128, 1], F16, kind="ExternalInput")
    iotasig_d = nc.dram_tensor("iotasig", [128, 1], F32, kind="ExternalInput")
    onehot_d = nc.dram_tensor("onehot", [SROWS, SROWS * 128], F16,
                              kind="ExternalInput")
    ident_d = nc.dram_tensor("ident", [H, H], F16, kind="ExternalInput")
    w_d = nc.dram_tensor("w", [L * H, H], F16, kind="ExternalInput")
    bias_d = nc.dram_tensor("bias", [L * H, 1], F32, kind="ExternalInput")
    out_d = nc.dram_tensor("out", [G_SLOTS, H], F32, kind="ExternalOutput")

    with tile.TileContext(nc) as tc:
        with tc.tile_pool(name="const", bufs=1) as cp, \
             tc.tile_pool(name="dram", bufs=1, space="DRAM") as dp, \
             tc.tile_pool(name="msg", bufs=2) as mp, \
             tc.tile_pool(name="sb", bufs=3) as sp, \
             tc.tile_pool(name="ps", bufs=2, space="PSUM") as pp, \
             tc.tile_pool(name="pool_ps", bufs=1, space="PSUM") as ppool:

            nc.gpsimd.load_library(_mlp_lib)

            iota_t = cp.tile([128, 1], F16)
            nc.sync.dma_start(iota_t[:], iota_d[:])
            iotasig_t = cp.tile([128, 1], F32)
            nc.sync.dma_start(iotasig_t[:], iotasig_d[:])
            ident_t = cp.tile([H, H], F16)
            nc.sync.dma_start(ident_t[:], ident_d[:])
            ones16 = cp.tile([1, 128], F16)
            nc.vector.memset(ones16[:], 1.0)
            onehot_t = cp.tile([SROWS, SROWS * 128], F16)
            nc.sync.dma_start(onehot_t[:], onehot_d[:])
            w_tiles, bias_tiles = [], []
            for l in range(L):
                wt = cp.tile([H, H], F16, tag=f"w{l}")
                nc.sync.dma_start(wt[:], w_d[l * H:(l + 1) * H, :])
                w_tiles.append(wt)
                bt = cp.tile([H, 1], F32, tag=f"b{l}")
                nc.sync.dma_start(bt[:], bias_d[l * H:(l + 1) * H, :])
                bias_tiles.append(bt)
            recip_t = cp.tile([G_SLOTS, 1], F32)
            nc.sync.dma_start(recip_t[:], recip_d[:])
            discolt_t = cp.tile([128, n_win], F32)
            nc.sync.dma_start(discolt_t[:], discolt_d[:])
            disrow_t = cp.tile([1, n_win * 128], F16)
            nc.sync.dma_start(disrow_t[:], disrow_d[:])
            cspool_t = cp.tile([128, n_win * G_SLOTS], F16)
            nc.sync.dma_start(cspool_t[:], cspool_d[:])
            # persistent: starts table + idx stream
            stg = cp.tile([SROWS, SROW_COLS], F16)
            nc.sync.dma_start(stg[:], starts_d[:])
            idxs_t = cp.tile([128, TOT_IDX // 16], I16)
            nc.sync.dma_start(idxs_t[:], idx_d[:])

            shard_a = dp.tile([S_pad, H], F16)
            shard_b = dp.tile([S_pad, H], F16)
            tspace = "Local" if single_core else "Shared"
            tbls = [[dp.tile([int(rows_q[q]), H], F16, addr_space=tspace,
                             name=f"tbl{l}_{q}") for q in range(NP)]
                    for l in range(L)]

            def transform(hT_f16, w_l, w, tq, qi):
                tps = pp.tile([WIN, H], F32, space="PSUM", tag="tps")
                nc.tensor.matmul(out=tps[:], lhsT=hT_f16[:, :WIN], rhs=w_l[:],
                                 start=True, stop=True)
                nc.scalar.activation(out=tq[:, qi, :], in_=tps[:],
                                     func=mybir.ActivationFunctionType.Copy,
                                     scale=discolt_t[0:WIN, w:w + 1])

            def flush_group(tq, grp, shard):
                gw0, glen = grp[0], len(grp)
                dst = shard[gw0 * WIN:(gw0 + glen) * WIN, :].rearrange(
                    "(q p) h -> p q h", p=WIN)
                nc.sync.dma_start(dst, tq[:, :glen, :])

            def emit_ag(q, shard, tbl_row):
                if single_core:
                    nc.sync.dma_start(
                        tbl_row[q][0:int(nw[q]) * WIN, :],
                        shard[int(w0[q]) * WIN:int(w0[q] + nw[q]) * WIN, :])
                else:
                    nc.gpsimd.collective_compute(
                        "AllGather", mybir.AluOpType.bypass,
                        replica_groups=[list(range(NCORES))],
                        ins=[shard[int(w0[q]) * WIN:int(w0[q] + nw[q]) * WIN, :]],
                        outs=[tbl_row[q][:]])

            # ---------------- layer 0 prologue: t0 = dis*(x@W0) --------------
            for gi, grp in enumerate(wgroups):
                tq = sp.tile([WIN, GMAX, H], F16, tag="t_sb")
                for qi, w in enumerate(grp):
                    xT = sp.tile([H, 128], F16, tag="xT")
                    nc.sync.dma_start(xT[:], xin_d[w * WIN:w * WIN + 128, :],
                                      transpose=True)
                    transform(xT, w_tiles[0], w, tq, qi)
                flush_group(tq, grp, shard_a)
                if grp_piece[gi] >= 0:
                    emit_ag(grp_piece[gi], shard_a, tbls[0])

            # ---------------- layers ----------------
            pool_ps = ppool.tile([G_SLOTS, H], F32, space="PSUM")
            n_pool_mm = 0

            for l in range(L):
                last = l == L - 1
                tbl_row = tbls[l]
                nshard = shard_b if l % 2 == 0 else shard_a
                ntbl = tbls[l + 1] if not last else None

                chunk_tiles = []
                for ci, (q, lo, hi) in enumerate(calls):
                    n = hi - lo
                    mt = mp.tile([128, n // 128, H], F16, tag=f"m{q}")
                    nc.gpsimd.dma_gather(
                        mt[:], tbl_row[q][:],
                        idxs_t[:, lo // 16:hi // 16], n, n, H,
                        single_packet=False, queue_num=ci % nq)
                    chunk_tiles.append((lo, hi, mt))

                def m_tile(p):
                    for lo, hi, mt in chunk_tiles:
                        if lo <= p < hi:
                            return mt[:, (p - lo) // 128, :]
                    raise AssertionError(p)

                csalt = 0
                for gi, grp in enumerate(wgroups):
                    cs_of = {}
                    for (r, cc, js) in cs_chunks[gi]:
                        k = len(js)
                        gn = k * 128
                        bps = pp.tile([128, 512], F32, space="PSUM", tag="bps")
                        nc.tensor.matmul(
                            out=bps[:, :gn],
                            lhsT=onehot_t[:, r * 128:(r + 1) * 128],
                            rhs=stg[0:SROWS, cc * 128:cc * 128 + gn],
                            start=True, stop=True)
                        cs = sp.tile([128, 512], F16, tag="cs")
                        if csalt % 2 == 0:
                            nc.vector.tensor_tensor(
                                out=cs[:, :gn],
                                in0=iota_t[:].to_broadcast([128, gn]),
                                in1=bps[:, :gn],
                                op=mybir.AluOpType.is_ge)
                        else:
                            nc.scalar.activation(
                                out=cs[:, :gn], in_=bps[:, :gn],
                                func=mybir.ActivationFunctionType.Sigmoid,
                                bias=iotasig_t[:], scale=-64.0)
                        csalt += 1
                        for ki, j in enumerate(js):
                            cs_of[j] = cs[:, ki * 128:(ki + 1) * 128]

                    tq = None
                    if not last:
                        tq = sp.tile([WIN, GMAX, H], F16, tag="t_sb")

                    for qi, w in enumerate(grp):
                        pairs = wt_pairs[w]
                        a_t = sp.tile([H, WIN], F32, tag="a")
                        if pairs:
                            suf = pp.tile([H, 128], F32, space="PSUM",
                                          tag="suf")
                            for k, (q, t) in enumerate(pairs):
                                nc.tensor.matmul(
                                    out=suf[:], lhsT=m_tile(t * 128),
                                    rhs=cs_of[pair_of[(w, q, t)]],
                                    start=(k == 0), stop=(k == len(pairs) - 1))
                            suf_sb = sp.tile([H, 128], F32, tag="suf_sb")
                            nc.scalar.activation(
                                out=suf_sb[:], in_=suf[:],
                                func=mybir.ActivationFunctionType.Copy)
                            nc.vector.tensor_tensor(out=a_t[:],
                                                    in0=suf_sb[:, :WIN],
                                                    in1=suf_sb[:, 1:WIN + 1],
                                                    op=mybir.AluOpType.subtract)
                        else:
                            nc.vector.memset(a_t[:], 0.0)

                        disb = pp.tile([128, WIN], F32, space="PSUM",
                                       tag="tps")
                        nc.tensor.matmul(out=disb[:], lhsT=ones16[:],
                                         rhs=disrow_t[:,
                                                      w * 128:w * 128 + WIN],
                                         start=True, stop=True)
                        u_t = sp.tile([H, WIN], F32, tag="u")
                        nc.vector.tensor_tensor(out=u_t[:], in0=a_t[:],
                                                in1=disb[:],
                                                op=mybir.AluOpType.mult)
                        hT = sp.tile([H, WIN], F16, tag="hT")
                        nc.scalar.activation(
                            out=hT[:], in_=u_t[:],
                            func=mybir.ActivationFunctionType.Relu,
                            bias=bias_tiles[l][:], scale=1.0)
                        if not last:
                            transform(hT, w_tiles[l + 1], w, tq, qi)
                        else:
                            hnm_ps = pp.tile([WIN, H], F32, space="PSUM",
                                             tag="tps")
                            nc.tensor.matmul(out=hnm_ps[:], lhsT=hT[:, :WIN],
                                             rhs=ident_t[:], start=True,
                                             stop=True)
                            hnm = sp.tile([WIN, H], F16, tag="hnm")
                            nc.vector.tensor_copy(hnm[:], hnm_ps[:])
                            nc.tensor.matmul(
                                out=pool_ps[:],
                                lhsT=cspool_t[0:WIN,
                                              w * G_SLOTS:(w + 1) * G_SLOTS],
                                rhs=hnm[:],
                                start=(n_pool_mm == 0),
                                stop=(n_pool_mm == n_win - 1))
                            n_pool_mm += 1
                    if not last:
                        flush_group(tq, grp, nshard)
                        if grp_piece[gi] >= 0:
                            emit_ag(grp_piece[gi], nshard, ntbl)

            pool_sb = sp.tile([G_SLOTS, H], F32, tag="pool_sb")
            nc.vector.tensor_scalar_mul(pool_sb[:], pool_ps[:], recip_t[:])
            nc.sync.dma_start(out_d[:], pool_sb[:])

    nc.compile()
    return nc


# --------------------------------------------------------------------------

def _full_in_maps(host, per_core, inputs):
    Ws = np.asarray(inputs["Ws"], dtype=np.float32)
    bs = np.asarray(inputs["bs"], dtype=np.float32)
    L, H = Ws.shape[0], Ws.shape[1]
    iota_col = np.arange(128, dtype=np.float16).reshape(128, 1)
    iotasig = (64.0 * np.arange(128) + 32.0).astype(np.float32).reshape(128, 1)
    ident = np.eye(H, dtype=np.float16)
    onehot = np.zeros((SROWS, SROWS * 128), dtype=np.float16)
    for r in range(SROWS):
        onehot[r, r * 128:(r + 1) * 128] = 1.0
    w_fp16 = np.ascontiguousarray(Ws.astype(np.float16).reshape(L * H, H))
    bias = np.ascontiguousarray(bs.astype(np.float32).reshape(L * H, 1))
    return [
        dict(pc, iota=iota_col, iotasig=iotasig, ident=ident, onehot=onehot,
             w=w_fp16, bias=bias)
        for pc in per_core
    ]


def kernel(x, edge_index, batch, Ws, bs):
    x = np.asarray(x)
    edge_index = np.asarray(edge_index)
    batch = np.asarray(batch)
    Ws = np.asarray(Ws, dtype=np.float32)
    bs = np.asarray(bs, dtype=np.float32)
    L, H = Ws.shape[0], Ws.shape[1]

    host, per_core = _build_host(x, edge_index, batch)
    nc = _build_program(host, L)
    in_maps = _full_in_maps(host, per_core, dict(Ws=Ws, bs=bs))
    res = run_bass_kernel_spmd(nc, in_maps, core_ids=list(range(NCORES)))

    G = host["G"]
    out = np.zeros((G, H), dtype=np.float32)
    for c in range(NCORES):
        fg = int(host["first_graph"][c])
        for g in host["g_of_core"][c]:
            out[g] = res.results[c]["out"][g - fg]
    return out
